# revision 8
# baseline (speedup 1.0000x reference)
"""Trainium2 Bass kernel for decode-step multi-head attention with RoPE
re-applied to the full KV cache (nn_MultiHeadAttention_50216757624897).

Sharding: 16 heads tensor-parallel across 8 cores (2 heads/core).
QKV weights split column-wise by head, KV cache split on the head dim,
out-proj row-parallel; partials summed on host (the unshard step).

Architecture (v2, transposed-K layout):
 - K cache host-permuted to [b, (h,d), s] and stored fp8-e3m4: partitions
   carry the 2x64 head-dims, the free dim carries all 4096 positions.
 - Scores: score[s,h] = sum_d K[(h,d),s] * E[(h,d),s] where the RoPE'd
   query E = cos~ (.) u + sin~ (.) v has u,v as PER-PARTITION scalars, so
   E builds with tensor_scalar ops that hit the DVE 4x_2p mode. The
   d-reduction runs on PE: per 128-position chunk one matmul with
   lhsT = P-chunk (stationary) and rhs = the [128,2] head-mask, writing
   scores [128 positions, 2 heads] straight into PSUM - no DVE reduce.
 - cos~ carries 1 on passthrough rows and sin~ carries 0, so the
   passthrough dims need no separate handling anywhere.
 - The new (current) token's K rotation cancels with Q's: score_new = qh.kh.
 - Softmax runs without max-subtraction (|score/8| < 3).
 - V cache host-permuted to [b, p, (h, c, d)] fp8-e3m4 (position = c*128+p)
   so A.V contracts over partitions exactly like the score layout; K and V
   ship as ONE concatenated DMA per batch.
 - Engine budget per batch: DVE t1+t2+Eadd[:3392] ~4.0us, Pool
   Eadd[3392:]+P ~4.0us, PE 32 score + 64 A.V matmuls, Act 2 exps.
"""

import sys
from contextlib import ExitStack

import numpy as np
import ml_dtypes

sys.path.insert(0, "/opt/trn_rl_repo")

import concourse.bass as bass
import concourse.bacc as bacc
import concourse.tile as tile
from concourse import mybir
from concourse.bass_types import AP
from concourse.bass_utils import run_bass_kernel_spmd

F32 = mybir.dt.float32
F16 = mybir.dt.float16
F8 = mybir.dt.float8e3
AF = mybir.ActivationFunctionType
AX = mybir.AxisListType

BS, NH, HD, ROT, CL, D = 8, 16, 64, 32, 4096, 1024
THETA = 10000.0
N_CORES = 8
H_PER_CORE = NH // N_CORES  # 2
ESPLIT = 3392  # Eadd column split: [0:ESPLIT] on DVE, rest on Pool


def _fap(t, off, dims):
    """AP over tile t with the tile's partition dim, extra free-dim spec."""
    b = t[:]
    return AP(tensor=b.tensor, offset=b.offset + off, ap=[list(b.ap[0])] + dims)


def _rotap(t, off):
    """[8, 2h, 16pairs] strided view of a [8,128] tile selecting pair elem
    `off` (0=even, 1=odd) of the rotary dims."""
    return _fap(t, off, [[64, 2], [2, 16]])


def build_program():
    nc = bacc.Bacc("TRN2", target_bir_lowering=False, debug=False)

    kv_c = nc.dram_tensor("kv_c", [BS, 2, 128, CL], F8, kind="ExternalInput")
    q_t = nc.dram_tensor("q_t", [D, BS], F16, kind="ExternalInput")
    wqkv_t = nc.dram_tensor("wqkv_t", [D, 384], F16, kind="ExternalInput")
    bqkv = nc.dram_tensor("bqkv", [1, 384], F16, kind="ExternalInput")
    wo_t = nc.dram_tensor("wo_t", [128, D], F16, kind="ExternalInput")
    cos_t = nc.dram_tensor("cos_t", [128, CL], F16, kind="ExternalInput")
    sin_t = nc.dram_tensor("sin_t", [128, CL], F16, kind="ExternalInput")
    cq_t = nc.dram_tensor("cq_t", [BS, 128], F32, kind="ExternalInput")
    sq_t = nc.dram_tensor("sq_t", [BS, 128], F32, kind="ExternalInput")
    id8 = nc.dram_tensor("id8", [8, 8], F32, kind="ExternalInput")
    id8f = nc.dram_tensor("id8f", [8, 8], F16, kind="ExternalInput")
    hmask = nc.dram_tensor("hmask", [128, 2], F16, kind="ExternalInput")
    out_p = nc.dram_tensor("out_p", [BS, D], F32, kind="ExternalOutput")

    with tile.TileContext(nc) as tc:
        with ExitStack() as ctx:
            _body(nc, tc, ctx, locals())
    nc.finalize()
    return nc


def _body(nc, tc, ctx, t):
    kv_c = t["kv_c"]
    out_p = t["out_p"]

    const = ctx.enter_context(tc.tile_pool(name="const", bufs=1))
    small = ctx.enter_context(tc.tile_pool(name="small", bufs=1))

    # ---- constants into SBUF. q/wqkv gate the q-chain; cos/sin gate the
    # per-batch E-build, so they go before the kv stream.
    sb_qt = const.tile([128, 8, 8], F16, tag="qt")
    nc.sync.dma_start(sb_qt[:], t["q_t"].rearrange("(c p) b -> p c b", p=128))
    sb_wqkv = const.tile([128, 8, 384], F16, tag="wqkv")
    nc.sync.dma_start(sb_wqkv[:], t["wqkv_t"].rearrange("(c p) n -> p c n", p=128))
    sb_cos = const.tile([128, CL], F16, tag="cos")
    nc.gpsimd.dma_start(sb_cos[:], t["cos_t"][:, :])
    sb_sin = const.tile([128, CL], F16, tag="sin")
    nc.gpsimd.dma_start(sb_sin[:], t["sin_t"][:, :])

    sb_bqkv = const.tile([1, 384], F16, tag="bqkv")
    nc.scalar.dma_start(sb_bqkv[:], t["bqkv"][:, :])
    sb_cq = const.tile([BS, 128], F32, tag="cq")
    nc.scalar.dma_start(sb_cq[:], t["cq_t"][:, :])
    sb_sq = const.tile([BS, 128], F32, tag="sq")
    nc.scalar.dma_start(sb_sq[:], t["sq_t"][:, :])
    sb_id8 = const.tile([8, 8], F32, tag="id8")
    nc.scalar.dma_start(sb_id8[:], t["id8"][:, :])
    sb_id8f = const.tile([8, 8], F16, tag="id8f")
    nc.scalar.dma_start(sb_id8f[:], t["id8f"][:, :])
    sb_hmask = const.tile([128, 2], F16, tag="hmask")
    nc.scalar.dma_start(sb_hmask[:], t["hmask"][:, :])
    sb_wo0 = const.tile([64, 1024], F16, tag="wo0")
    nc.scalar.dma_start(sb_wo0[:], t["wo_t"][0:64, :])
    sb_wo1 = const.tile([64, 1024], F16, tag="wo1")
    nc.scalar.dma_start(sb_wo1[:], t["wo_t"][64:128, :])

    ones_p = const.tile([128, 1], F32, tag="ones_p")
    nc.vector.memset(ones_p[:], 1.0)
    ones_r8 = const.tile([1, 8], F16, tag="ones_r8")
    nc.vector.memset(ones_r8[:], 1.0)
    ones_r64 = const.tile([1, 64], F32, tag="ones_r64")
    nc.vector.memset(ones_r64[:], 1.0)

    # ---- q/k/v projection of the new token
    qtr_stack = ExitStack()
    psum_proj = qtr_stack.enter_context(tc.tile_pool(name="psum_proj", bufs=1, space="PSUM"))
    projs = small.tile([8, 384], F32, tag="projs")
    ps_qkv = psum_proj.tile([8, 384], F32, tag="ps_qkv")
    ps_q = ps_qkv[:, 0:128]
    for ci in range(8):
        nc.tensor.matmul(ps_q, lhsT=sb_qt[:, ci, :], rhs=sb_wqkv[:, ci, 0:128],
                         start=(ci == 0), stop=False, skip_group_check=True)
    nc.tensor.matmul(ps_q, lhsT=ones_r8[:], rhs=sb_bqkv[:, 0:128],
                     start=False, stop=True, skip_group_check=True)
    nc.scalar.copy(projs[:, 0:128], ps_q)
    ps_kv = ps_qkv[:, 128:384]
    for ci in range(8):
        nc.tensor.matmul(ps_kv, lhsT=sb_qt[:, ci, :], rhs=sb_wqkv[:, ci, 128:384],
                         start=False, stop=False, skip_group_check=True)
    nc.tensor.matmul(ps_kv, lhsT=ones_r8[:], rhs=sb_bqkv[:, 128:384],
                     start=False, stop=True, skip_group_check=True)
    nc.scalar.copy(projs[:, 128:384], ps_kv)
    qh, kh, vh = projs[:, 0:128], projs[:, 128:256], projs[:, 256:384]

    # ---- RoPE on q (full width: host tables carry [cos|1], [sin|0]); u and
    # v = G(u) side by side in one [8, 256] f16 tile.
    qrv = small.tile([8, 256], F16, tag="qrv")
    qr, vG = qrv[:, 0:128], qrv[:, 128:256]
    Hh = small.tile([8, 128], F32, tag="Hh")
    nc.vector.memset(Hh[:], 0.0)
    nc.vector.tensor_scalar_mul(_rotap(Hh, 0), _fap(ps_q, 1, [[64, 2], [2, 16]]), -1.0)
    nc.vector.tensor_copy(_rotap(Hh, 1), _fap(ps_q, 0, [[64, 2], [2, 16]]))
    t1q = small.tile([8, 128], F32, tag="t1q")
    nc.vector.tensor_mul(t1q[:], ps_q[:], sb_cq[:])
    t2q = small.tile([8, 128], F32, tag="t2q")
    nc.vector.tensor_mul(t2q[:], Hh[:], sb_sq[:])
    nc.vector.tensor_add(qr, t2q[:], t1q[:])
    # v = G(q_rot): pairs (x0,x1) -> (x1,-x0); zero elsewhere
    nc.vector.memset(vG, 0.0)
    nc.vector.tensor_copy(_fap(qrv, 128, [[64, 2], [2, 16]]),
                          _fap(qrv, 1, [[64, 2], [2, 16]]))
    nc.vector.tensor_scalar_mul(_fap(qrv, 129, [[64, 2], [2, 16]]),
                                _fap(qrv, 0, [[64, 2], [2, 16]]), -1.0)

    # ---- transpose u, v to per-partition layout [128 (h,d), 8 b]
    psum_tr = qtr_stack.enter_context(tc.tile_pool(name="psum_tr", bufs=1, space="PSUM"))
    uv_ps = psum_tr.tile([128, 16], F16, tag="uv_ps")
    nc.tensor.matmul(uv_ps[:, 0:8], lhsT=qr, rhs=sb_id8f[:], is_transpose=True,
                     start=True, stop=False, skip_group_check=True)
    nc.tensor.matmul(uv_ps[:, 8:16], lhsT=vG, rhs=sb_id8f[:], is_transpose=True,
                     start=False, stop=True, skip_group_check=True)
    u_T = small.tile([128, 8], F32, tag="u_T")
    nc.scalar.copy(u_T[:], uv_ps[:, 0:8])
    v_T = small.tile([128, 8], F32, tag="v_T")
    nc.scalar.copy(v_T[:], uv_ps[:, 8:16])

    # ---- new-token score: rotations cancel -> qh . kh
    sn = small.tile([8, 128], F32, tag="sn")
    nc.vector.tensor_mul(sn[:], qh, kh)
    scn = small.tile([8, 2], F32, tag="scn")
    nc.vector.reduce_sum(scn[:], _fap(sn, 0, [[64, 2], [1, 64]]), axis=AX.X)
    expn = small.tile([8, 2], F32, tag="expn")
    nc.scalar.activation(expn[:], scn[:], AF.Exp, scale=0.125)
    vhs = small.tile([8, 128], F32, tag="vhs")
    nc.vector.tensor_mul(_fap(vhs, 0, [[64, 2], [1, 64]]),
                         _fap(projs, 256, [[64, 2], [1, 64]]),
                         _fap(expn, 0, [[1, 2], [0, 64]]))

    qtr_stack.close()  # release proj/transpose PSUM banks for the loop pools

    # ---- main per-batch loop
    kvpool = ctx.enter_context(tc.tile_pool(name="kvpool", bufs=3))
    epool = ctx.enter_context(tc.tile_pool(name="epool", bufs=2))
    Ppool = ctx.enter_context(tc.tile_pool(name="Ppool", bufs=2))
    apool = ctx.enter_context(tc.tile_pool(name="apool", bufs=3))
    psum_sc = ctx.enter_context(tc.tile_pool(name="psum_sc", bufs=2, space="PSUM"))
    psum_r = ctx.enter_context(tc.tile_pool(name="psum_r", bufs=1, space="PSUM"))
    psum_wo = ctx.enter_context(tc.tile_pool(name="psum_wo", bufs=2, space="PSUM"))
    psum_main = ctx.enter_context(tc.tile_pool(name="psum_main", bufs=1, space="PSUM"))

    ov_ps = psum_main.tile([64, 16], F32, tag="ov")
    den_ps = psum_main.tile([1, 16], F32, tag="den")
    den_part = small.tile([128, 16], F32, tag="den_part")

    # init PSUM with the new-token contribution (transposes of vh*exp, exp)
    # NOTE: PSUM start=True zeroes the whole 2KB bank row, so only the FIRST
    # write into each psum tile may use start=True.
    for h in range(H_PER_CORE):
        nc.tensor.matmul(ov_ps[:, h * 8:(h + 1) * 8], lhsT=vhs[:, h * 64:(h + 1) * 64],
                         rhs=sb_id8[:], is_transpose=True, start=(h == 0), stop=False,
                         skip_group_check=True)
        nc.tensor.matmul(den_ps[:, h * 8:(h + 1) * 8], lhsT=expn[:, h:h + 1],
                         rhs=sb_id8[:], is_transpose=True, start=(h == 0), stop=False,
                         skip_group_check=True)

    def b_iter(b):
        kvt = kvpool.tile([128, 2 * CL], F8, tag="kv")
        kvsrc = kv_c[b]
        kv_eng = nc.sync if b % 2 == 0 else nc.scalar
        kv_eng.dma_start(kvt[:], AP(tensor=kvsrc.tensor, offset=kvsrc.offset,
                                    ap=[[CL, 128], [128 * CL, 2], [1, CL]]))
        kt, voff = kvt[:, 0:CL], CL

        # E = cos~*u + sin~*v: tensor_scalar ops at 4x, add split DVE/Pool
        E = epool.tile([128, CL], F16, tag="E")
        nc.vector.tensor_scalar(E[:], sb_cos[:], u_T[:, b:b + 1], None,
                                mybir.AluOpType.mult)
        T2 = epool.tile([128, CL], F16, tag="T2")
        nc.vector.tensor_scalar(T2[:], sb_sin[:], v_T[:, b:b + 1], None,
                                mybir.AluOpType.mult)
        nc.vector.tensor_add(E[:], E[:], T2[:])

        # P = k .* E  (fp8 x fp16 -> fp16, Pool)
        Pt = Ppool.tile([128, CL], F16, tag="P")
        nc.gpsimd.tensor_mul(Pt[:], kt, E[:])

        # scores: per 128-position chunk, one matmul contracting the 128
        # (h,d)-partitions against the head mask -> [128 pos, 2 heads]
        sc = psum_sc.tile([128, 64], F32, tag="sc", name=f"sc{b}")
        for c in range(32):
            nc.tensor.matmul(sc[:, 2 * c:2 * c + 2],
                             lhsT=Pt[:, c * 128:(c + 1) * 128], rhs=sb_hmask[:],
                             start=(c == 0), stop=(c == 31), skip_group_check=True)

        # exp + denominators; at cols (32h + c) <- sc cols (2c + h)
        at = apool.tile([128, 64], F16, tag="at")
        for h in range(H_PER_CORE):
            col = h * 8 + b
            scv = _fap(sc, h, [[2, 32]])
            nc.scalar.activation(at[:, h * 32:(h + 1) * 32], scv,
                                 AF.Exp, scale=0.125,
                                 accum_out=den_part[:, col:col + 1])
            for c in range(32):
                nc.tensor.matmul(ov_ps[:, col:col + 1],
                                 lhsT=_fap(kvt, voff + h * 2048 + c * 64, [[1, 64]]),
                                 rhs=at[:, h * 32 + c:h * 32 + c + 1],
                                 start=False, stop=(c == 31), skip_group_check=True)

    for b in range(8):
        b_iter(b)

    # denominator: column-sum of per-partition exp sums + new-token init
    nc.tensor.matmul(den_ps[:], lhsT=ones_p[:], rhs=den_part[:],
                     start=False, stop=True, skip_group_check=True)

    # ---- normalize + out-projection
    ov_sb = small.tile([64, 16], F32, tag="ov_sb")
    nc.scalar.copy(ov_sb[:], ov_ps[:])
    r_row = small.tile([1, 16], F32, tag="r_row")
    nc.vector.reciprocal(r_row[:], den_ps[:])
    r_ps = psum_r.tile([64, 16], F32, tag="r")
    nc.tensor.matmul(r_ps[:], lhsT=ones_r64[:], rhs=r_row[:], start=True, stop=True)
    on = small.tile([64, 16], F16, tag="on")
    nc.vector.tensor_mul(on[:], ov_sb[:], r_ps[:])

    out_f = small.tile([8, 1024], F32, tag="out_f")
    for nchunk in range(2):
        sl = slice(nchunk * 512, (nchunk + 1) * 512)
        ps = psum_wo.tile([8, 512], F32, tag="wo", name=f"wo_ps{nchunk}")
        nc.tensor.matmul(ps[:], lhsT=on[:, 0:8], rhs=sb_wo0[:, sl], start=True, stop=False)
        nc.tensor.matmul(ps[:], lhsT=on[:, 8:16], rhs=sb_wo1[:, sl], start=False, stop=True)
        nc.scalar.copy(out_f[:, sl], ps[:])
    nc.sync.dma_start(out_p[:, :], out_f[:])


def _host_tables():
    """cos~/sin~ in transposed layout [128 (h,d), 4096 s] plus q-side tables."""
    inv_freq = 1.0 / (THETA ** (np.arange(0, ROT, 2, dtype=np.float64) / ROT))
    invf_rep = np.repeat(inv_freq, 2)  # [32]
    pos = np.arange(CL, dtype=np.float64)
    ang = invf_rep[:, None] * pos[None, :]  # [32 rot-d, 4096 s]
    cos_h = np.concatenate([np.cos(ang), np.ones((32, CL))], axis=0)  # [64, 4096]
    sin_h = np.concatenate([np.sin(ang), np.zeros((32, CL))], axis=0)
    cos_t = np.concatenate([cos_h, cos_h], axis=0).astype(np.float16)  # [128, 4096]
    sin_t = np.concatenate([sin_h, sin_h], axis=0).astype(np.float16)
    fq = 4096.0 * invf_rep
    cq_row = np.concatenate([np.cos(fq), np.ones(32)])  # per head [64]
    sq_row = np.concatenate([np.sin(fq), np.zeros(32)])
    cq_t = np.tile(np.concatenate([cq_row, cq_row]), (BS, 1)).astype(np.float32)
    sq_t = np.tile(np.concatenate([sq_row, sq_row]), (BS, 1)).astype(np.float32)
    return cos_t, sin_t, cq_t, sq_t


_NC = None


def _get_nc():
    global _NC
    if _NC is None:
        _NC = build_program()
    return _NC


def kernel(q, k_cache, v_cache, WQ_w, WQ_b, WK_w, WK_b, WV_w, WV_b, WO_w, WO_b,
           _trace=False, _tmpdir=None):
    q = np.asarray(q, dtype=np.float32)
    k8 = np.asarray(k_cache, dtype=np.float32).astype(ml_dtypes.float8_e3m4)
    v8 = np.asarray(v_cache, dtype=np.float32).astype(ml_dtypes.float8_e3m4)
    cos_t, sin_t, cq_t, sq_t = _host_tables()
    q_t = np.ascontiguousarray(q.reshape(BS, D).T.astype(np.float16))
    id8 = np.eye(8, dtype=np.float32)
    id8f = np.eye(8, dtype=np.float16)
    hmask = np.zeros((128, 2), np.float16)
    hmask[0:64, 0] = 1.0
    hmask[64:128, 1] = 1.0

    in_maps = []
    for c in range(N_CORES):
        sl = slice(c * 128, (c + 1) * 128)
        hs = slice(c * H_PER_CORE, (c + 1) * H_PER_CORE)
        # K: [b,h,s,d] -> [b, (h d), s]
        kc = k8[:, hs].transpose(0, 1, 3, 2).reshape(BS, 128, CL)
        # V: [b,h,s,d] -> [b, p, (h c d)] with s = c*128 + p
        vc = v8[:, hs].reshape(BS, H_PER_CORE, 32, 128, HD)
        vc = vc.transpose(0, 3, 1, 2, 4).reshape(BS, 128, CL)
        kv = np.stack([kc, vc], axis=1)  # [b, 2, 128, 4096]
        in_maps.append({
            "kv_c": np.ascontiguousarray(kv),
            "q_t": q_t,
            "wqkv_t": np.ascontiguousarray(np.concatenate(
                [np.asarray(WQ_w, np.float32)[sl].T,
                 np.asarray(WK_w, np.float32)[sl].T,
                 np.asarray(WV_w, np.float32)[sl].T], axis=1).astype(np.float16)),
            "bqkv": np.ascontiguousarray(np.concatenate(
                [np.asarray(WQ_b, np.float32)[sl],
                 np.asarray(WK_b, np.float32)[sl],
                 np.asarray(WV_b, np.float32)[sl]]).reshape(1, 384).astype(np.float16)),
            "wo_t": np.ascontiguousarray(
                np.asarray(WO_w, np.float32)[:, sl].T.astype(np.float16)),
            "cos_t": cos_t, "sin_t": sin_t, "cq_t": cq_t, "sq_t": sq_t,
            "id8": id8, "id8f": id8f, "hmask": hmask,
        })

    nc = _get_nc()
    res = run_bass_kernel_spmd(nc, in_maps, list(range(N_CORES)),
                               trace=_trace, tmpdir=_tmpdir)
    partials = [np.asarray(res.results[c]["out_p"], dtype=np.float64)
                for c in range(N_CORES)]
    out = np.sum(partials, axis=0) + np.asarray(WO_b, np.float64)
    if _trace:
        kernel._last_results = res
    return out.reshape(BS, 1, D).astype(np.float32)


# revision 14
# speedup vs baseline: 1.4270x; 1.4270x over previous
"""Trainium2 Bass kernel for decode-step multi-head attention with RoPE
re-applied to the full KV cache (nn_MultiHeadAttention_50216757624897).

Sharding: 16 heads tensor-parallel across 8 cores (2 heads/core).
QKV weights split column-wise by head, KV cache split on the head dim,
out-proj row-parallel; partials summed on host (the unshard step).

Architecture (v3, transposed-K + 2-pass score fold):
 - K cache host-permuted to [b, (h,d), s] fp16: partitions carry the 2x64
   head-dims, the free dim carries all 4096 positions.
 - score[s,h] = sum_d k*cos~*u + sum_d k*sin~*v. Only the two products
   kc = k (.) cos~ and ks = k (.) sin~ are elementwise (DVE 2x / Pool);
   the (.) u / (.) v and the d-reduction fold into PE: per 128-position
   chunk, two accumulating matmuls with stationary lhsT = kc/ks chunks
   and rhs = tiny per-batch masks (hmask * u_b / * v_b), writing scores
   [128 positions, 2 heads] straight to PSUM. No E tile, no DVE reduce.
 - cos~ rows are 1 and sin~ rows are 0 on passthrough dims, so those need
   no separate handling (ks passthrough contributes v*0).
 - The new (current) token's K rotation cancels with Q's: score_new = qh.kh.
 - Softmax runs without max-subtraction (|score/8| < 3).
 - V cache host-permuted to [b, p, (h, c, d)] fp8-e3m4 (position = c*128+p):
   it is consumed only by PE A.V matmuls (fp8 full-rate), halving its HBM
   traffic; A.V contracts over partitions like the score layout.
 - DMA transfers overlap across issuing queues in the cost model; the kv
   stream alternates SP/Act and the cos/sin tables ship as halves on
   SWDGE/Act so both tables land by ~6us.
"""

import os
import sys
from contextlib import ExitStack

import numpy as np
import ml_dtypes

sys.path.insert(0, "/opt/trn_rl_repo")

import concourse.bass as bass
import concourse.bacc as bacc
import concourse.tile as tile
from concourse import mybir
from concourse.bass_types import AP
from concourse.bass_utils import run_bass_kernel_spmd

F32 = mybir.dt.float32
F16 = mybir.dt.float16
F8 = mybir.dt.float8e3
AF = mybir.ActivationFunctionType
AX = mybir.AxisListType
OP = mybir.AluOpType

BS, NH, HD, ROT, CL, D = 8, 16, 64, 32, 4096, 1024
THETA = 10000.0
N_CORES = 8
H_PER_CORE = NH // N_CORES  # 2
HALF = CL // 2

# ks half-products steered to DVE for these batches (engine balance knob)
KS_DVE = set(int(x) for x in os.environ.get("KS_DVE", "2,5").split(",") if x != "")


def _fap(t, off, dims):
    """AP over tile t with the tile's partition dim, extra free-dim spec."""
    b = t[:]
    return AP(tensor=b.tensor, offset=b.offset + off, ap=[list(b.ap[0])] + dims)


def _rotap(t, off):
    """[8, 2h, 16pairs] strided view of a [8,128] tile selecting pair elem
    `off` (0=even, 1=odd) of the rotary dims."""
    return _fap(t, off, [[64, 2], [2, 16]])


def build_program():
    nc = bacc.Bacc("TRN2", target_bir_lowering=False, debug=False)

    k_c = nc.dram_tensor("k_c", [BS, 128, CL], F16, kind="ExternalInput")
    v_c = nc.dram_tensor("v_c", [BS, 128, CL], F8, kind="ExternalInput")
    q_t = nc.dram_tensor("q_t", [D, BS], F16, kind="ExternalInput")
    wqkv_t = nc.dram_tensor("wqkv_t", [D, 384], F16, kind="ExternalInput")
    bqkv = nc.dram_tensor("bqkv", [1, 384], F16, kind="ExternalInput")
    wo_t = nc.dram_tensor("wo_t", [128, D], F16, kind="ExternalInput")
    cos_t = nc.dram_tensor("cos_t", [128, CL], F16, kind="ExternalInput")
    sin_t = nc.dram_tensor("sin_t", [128, CL], F16, kind="ExternalInput")
    cq_t = nc.dram_tensor("cq_t", [BS, 128], F32, kind="ExternalInput")
    sq_t = nc.dram_tensor("sq_t", [BS, 128], F32, kind="ExternalInput")
    id8 = nc.dram_tensor("id8", [8, 8], F32, kind="ExternalInput")
    id8f = nc.dram_tensor("id8f", [8, 8], F16, kind="ExternalInput")
    hmask = nc.dram_tensor("hmask", [128, 2], F16, kind="ExternalInput")
    out_p = nc.dram_tensor("out_p", [BS, D], F32, kind="ExternalOutput")

    with tile.TileContext(nc) as tc:
        with ExitStack() as ctx:
            _body(nc, tc, ctx, locals())
    nc.finalize()
    return nc


def _body(nc, tc, ctx, t):
    k_c, v_c = t["k_c"], t["v_c"]
    out_p = t["out_p"]

    const = ctx.enter_context(tc.tile_pool(name="const", bufs=1))
    small = ctx.enter_context(tc.tile_pool(name="small", bufs=1))

    # ---- constants. q/wqkv gate the q-chain (SP); table halves spread over
    # SWDGE + Act so both tables land by ~6us; kv stream follows.
    sb_qt = const.tile([128, 8, 8], F16, tag="qt")
    nc.sync.dma_start(sb_qt[:], t["q_t"].rearrange("(c p) b -> p c b", p=128))
    sb_wqkv = const.tile([128, 8, 384], F16, tag="wqkv")
    nc.sync.dma_start(sb_wqkv[:], t["wqkv_t"].rearrange("(c p) n -> p c n", p=128))

    sb_cos = const.tile([128, CL], F16, tag="cos")
    sb_sin = const.tile([128, CL], F16, tag="sin")
    nc.gpsimd.dma_start(sb_cos[:, 0:HALF], t["cos_t"][:, 0:HALF])
    nc.scalar.dma_start(sb_cos[:, HALF:CL], t["cos_t"][:, HALF:CL])
    nc.gpsimd.dma_start(sb_sin[:, 0:HALF], t["sin_t"][:, 0:HALF])
    nc.scalar.dma_start(sb_sin[:, HALF:CL], t["sin_t"][:, HALF:CL])

    sb_bqkv = const.tile([1, 384], F16, tag="bqkv")
    nc.scalar.dma_start(sb_bqkv[:], t["bqkv"][:, :])
    sb_cq = const.tile([BS, 128], F32, tag="cq")
    nc.scalar.dma_start(sb_cq[:], t["cq_t"][:, :])
    sb_sq = const.tile([BS, 128], F32, tag="sq")
    nc.scalar.dma_start(sb_sq[:], t["sq_t"][:, :])
    sb_id8 = const.tile([8, 8], F32, tag="id8")
    nc.scalar.dma_start(sb_id8[:], t["id8"][:, :])
    sb_id8f = const.tile([8, 8], F16, tag="id8f")
    nc.scalar.dma_start(sb_id8f[:], t["id8f"][:, :])
    sb_hmask = const.tile([128, 2], F16, tag="hmask")
    nc.scalar.dma_start(sb_hmask[:], t["hmask"][:, :])
    sb_wo0 = const.tile([64, 1024], F16, tag="wo0")
    nc.scalar.dma_start(sb_wo0[:], t["wo_t"][0:64, :])
    sb_wo1 = const.tile([64, 1024], F16, tag="wo1")
    nc.scalar.dma_start(sb_wo1[:], t["wo_t"][64:128, :])

    ones_p = const.tile([128, 1], F32, tag="ones_p")
    nc.vector.memset(ones_p[:], 1.0)
    ones_r8 = const.tile([1, 8], F16, tag="ones_r8")
    nc.vector.memset(ones_r8[:], 1.0)
    ones_r64 = const.tile([1, 64], F32, tag="ones_r64")
    nc.vector.memset(ones_r64[:], 1.0)

    # ---- q/k/v projection of the new token
    qtr_stack = ExitStack()
    psum_proj = qtr_stack.enter_context(tc.tile_pool(name="psum_proj", bufs=1, space="PSUM"))
    projs = small.tile([8, 384], F32, tag="projs")
    ps_qkv = psum_proj.tile([8, 384], F32, tag="ps_qkv")
    ps_q = ps_qkv[:, 0:128]
    for ci in range(8):
        nc.tensor.matmul(ps_q, lhsT=sb_qt[:, ci, :], rhs=sb_wqkv[:, ci, 0:128],
                         start=(ci == 0), stop=False, skip_group_check=True)
    nc.tensor.matmul(ps_q, lhsT=ones_r8[:], rhs=sb_bqkv[:, 0:128],
                     start=False, stop=True, skip_group_check=True)
    nc.scalar.copy(projs[:, 0:128], ps_q)
    ps_kv = ps_qkv[:, 128:384]
    for ci in range(8):
        nc.tensor.matmul(ps_kv, lhsT=sb_qt[:, ci, :], rhs=sb_wqkv[:, ci, 128:384],
                         start=False, stop=False, skip_group_check=True)
    nc.tensor.matmul(ps_kv, lhsT=ones_r8[:], rhs=sb_bqkv[:, 128:384],
                     start=False, stop=True, skip_group_check=True)
    nc.scalar.copy(projs[:, 128:384], ps_kv)
    qh, kh, vh = projs[:, 0:128], projs[:, 128:256], projs[:, 256:384]

    # ---- RoPE on q (full width: host tables carry [cos|1], [sin|0]); u and
    # v = G(u) side by side in one [8, 256] f16 tile.
    qrv = small.tile([8, 256], F16, tag="qrv")
    qr, vG = qrv[:, 0:128], qrv[:, 128:256]
    Hh = small.tile([8, 128], F32, tag="Hh")
    nc.vector.memset(Hh[:], 0.0)
    nc.vector.tensor_scalar_mul(_rotap(Hh, 0), _fap(ps_qkv, 1, [[64, 2], [2, 16]]), -1.0)
    nc.vector.tensor_copy(_rotap(Hh, 1), _fap(ps_qkv, 0, [[64, 2], [2, 16]]))
    t1q = small.tile([8, 128], F32, tag="t1q")
    nc.vector.tensor_mul(t1q[:], ps_q, sb_cq[:])
    t2q = small.tile([8, 128], F32, tag="t2q")
    nc.vector.tensor_mul(t2q[:], Hh[:], sb_sq[:])
    nc.vector.tensor_add(qr, t2q[:], t1q[:])
    # v = G(q_rot): pairs (x0,x1) -> (x1,-x0); zero elsewhere
    nc.vector.memset(vG, 0.0)
    nc.vector.tensor_copy(_fap(qrv, 128, [[64, 2], [2, 16]]),
                          _fap(qrv, 1, [[64, 2], [2, 16]]))
    nc.vector.tensor_scalar_mul(_fap(qrv, 129, [[64, 2], [2, 16]]),
                                _fap(qrv, 0, [[64, 2], [2, 16]]), -1.0)

    # ---- transpose u, v to per-partition layout [128 (h,d), 8 b]
    psum_tr = qtr_stack.enter_context(tc.tile_pool(name="psum_tr", bufs=1, space="PSUM"))
    uv_ps = psum_tr.tile([128, 16], F16, tag="uv_ps")
    nc.tensor.matmul(uv_ps[:, 0:8], lhsT=qr, rhs=sb_id8f[:], is_transpose=True,
                     start=True, stop=False, skip_group_check=True)
    nc.tensor.matmul(uv_ps[:, 8:16], lhsT=vG, rhs=sb_id8f[:], is_transpose=True,
                     start=False, stop=True, skip_group_check=True)
    u_T = small.tile([128, 8], F32, tag="u_T")
    nc.scalar.copy(u_T[:], uv_ps[:, 0:8])
    v_T = small.tile([128, 8], F32, tag="v_T")
    nc.scalar.copy(v_T[:], uv_ps[:, 8:16])

    # per-batch score-mask tiles: umask[:, (b,h)] = hmask[:, h] * u_T[:, b]
    um = small.tile([128, 16], F16, tag="um")
    vm = small.tile([128, 16], F16, tag="vm")
    for b in range(8):
        nc.vector.tensor_scalar(um[:, 2 * b:2 * b + 2], sb_hmask[:],
                                u_T[:, b:b + 1], None, OP.mult)
        nc.vector.tensor_scalar(vm[:, 2 * b:2 * b + 2], sb_hmask[:],
                                v_T[:, b:b + 1], None, OP.mult)

    # ---- new-token score: rotations cancel -> qh . kh
    sn = small.tile([8, 128], F32, tag="sn")
    nc.vector.tensor_mul(sn[:], qh, kh)
    scn = small.tile([8, 2], F32, tag="scn")
    nc.vector.reduce_sum(scn[:], _fap(sn, 0, [[64, 2], [1, 64]]), axis=AX.X)
    expn = small.tile([8, 2], F32, tag="expn")
    nc.scalar.activation(expn[:], scn[:], AF.Exp, scale=0.125)
    vhs = small.tile([8, 128], F32, tag="vhs")
    nc.vector.tensor_mul(_fap(vhs, 0, [[64, 2], [1, 64]]),
                         _fap(projs, 256, [[64, 2], [1, 64]]),
                         _fap(expn, 0, [[1, 2], [0, 64]]))

    qtr_stack.close()  # release proj/transpose PSUM banks for the loop pools

    # ---- main per-batch loop
    kpool = ctx.enter_context(tc.tile_pool(name="kpool", bufs=3))
    vpool = ctx.enter_context(tc.tile_pool(name="vpool", bufs=3))
    kcpool = ctx.enter_context(tc.tile_pool(name="kcpool", bufs=2))
    kspool = ctx.enter_context(tc.tile_pool(name="kspool", bufs=2))
    apool = ctx.enter_context(tc.tile_pool(name="apool", bufs=3))
    psum_sc = ctx.enter_context(tc.tile_pool(name="psum_sc", bufs=3, space="PSUM"))
    psum_r = ctx.enter_context(tc.tile_pool(name="psum_r", bufs=1, space="PSUM"))
    psum_wo = ctx.enter_context(tc.tile_pool(name="psum_wo", bufs=2, space="PSUM"))
    psum_main = ctx.enter_context(tc.tile_pool(name="psum_main", bufs=1, space="PSUM"))

    ov_ps = psum_main.tile([64, 16], F32, tag="ov")
    den_ps = psum_main.tile([1, 16], F32, tag="den")
    den_part = small.tile([128, 16], F32, tag="den_part")

    # init PSUM with the new-token contribution (transposes of vh*exp, exp)
    # NOTE: PSUM start=True zeroes the whole 2KB bank row, so only the FIRST
    # write into each psum tile may use start=True.
    for h in range(H_PER_CORE):
        nc.tensor.matmul(ov_ps[:, h * 8:(h + 1) * 8], lhsT=vhs[:, h * 64:(h + 1) * 64],
                         rhs=sb_id8[:], is_transpose=True, start=(h == 0), stop=False,
                         skip_group_check=True)
        nc.tensor.matmul(den_ps[:, h * 8:(h + 1) * 8], lhsT=expn[:, h:h + 1],
                         rhs=sb_id8[:], is_transpose=True, start=(h == 0), stop=False,
                         skip_group_check=True)

    def b_iter(b):
        kt = kpool.tile([128, CL], F16, tag="k")
        vt = vpool.tile([128, CL], F8, tag="v")
        kv_eng = nc.sync if b % 2 == 0 else nc.scalar
        kv_eng.dma_start(kt[:, 0:HALF], k_c[b][:, 0:HALF])
        kv_eng.dma_start(kt[:, HALF:CL], k_c[b][:, HALF:CL])
        kv_eng.dma_start(vt[:], v_c[b])

        # kc = k (.) cos~, ks = k (.) sin~, in col-halves
        kc = kcpool.tile([128, CL], F16, tag="kc")
        ks = kspool.tile([128, CL], F16, tag="ks")
        sc = psum_sc.tile([128, 64], F32, tag="sc", name=f"sc{b}")
        for half in range(2):
            lo, hi = half * HALF, (half + 1) * HALF
            nc.vector.tensor_mul(kc[:, lo:hi], kt[:, lo:hi], sb_cos[:, lo:hi])
            ks_eng = nc.vector if (b in KS_DVE and half == 0) else nc.gpsimd
            ks_eng.tensor_mul(ks[:, lo:hi], kt[:, lo:hi], sb_sin[:, lo:hi])
            for c in range(half * 16, half * 16 + 16):
                nc.tensor.matmul(sc[:, 2 * c:2 * c + 2],
                                 lhsT=kc[:, c * 128:(c + 1) * 128],
                                 rhs=um[:, 2 * b:2 * b + 2],
                                 start=(c == 0), stop=False, skip_group_check=True)
                nc.tensor.matmul(sc[:, 2 * c:2 * c + 2],
                                 lhsT=ks[:, c * 128:(c + 1) * 128],
                                 rhs=vm[:, 2 * b:2 * b + 2],
                                 start=False, stop=(c == 31), skip_group_check=True)

        # exp + denominators; at cols (32h + c) <- sc cols (2c + h)
        at = apool.tile([128, 64], F16, tag="at")
        for h in range(H_PER_CORE):
            col = h * 8 + b
            scv = _fap(sc, h, [[2, 32]])
            nc.scalar.activation(at[:, h * 32:(h + 1) * 32], scv,
                                 AF.Exp, scale=0.125,
                                 accum_out=den_part[:, col:col + 1])
            for c in range(32):
                nc.tensor.matmul(ov_ps[:, col:col + 1],
                                 lhsT=_fap(vt, h * 2048 + c * 64, [[1, 64]]),
                                 rhs=at[:, h * 32 + c:h * 32 + c + 1],
                                 start=False, stop=(c == 31), skip_group_check=True)

    for b in range(8):
        b_iter(b)

    # denominator: column-sum of per-partition exp sums + new-token init
    nc.tensor.matmul(den_ps[:], lhsT=ones_p[:], rhs=den_part[:],
                     start=False, stop=True, skip_group_check=True)

    # ---- normalize + out-projection
    ov_sb = small.tile([64, 16], F32, tag="ov_sb")
    nc.scalar.copy(ov_sb[:], ov_ps[:])
    r_row = small.tile([1, 16], F32, tag="r_row")
    nc.vector.reciprocal(r_row[:], den_ps[:])
    r_ps = psum_r.tile([64, 16], F32, tag="r")
    nc.tensor.matmul(r_ps[:], lhsT=ones_r64[:], rhs=r_row[:], start=True, stop=True)
    on = small.tile([64, 16], F16, tag="on")
    nc.vector.tensor_mul(on[:], ov_sb[:], r_ps[:])

    out_f = small.tile([8, 1024], F32, tag="out_f")
    for nchunk in range(2):
        sl = slice(nchunk * 512, (nchunk + 1) * 512)
        ps = psum_wo.tile([8, 512], F32, tag="wo", name=f"wo_ps{nchunk}")
        nc.tensor.matmul(ps[:], lhsT=on[:, 0:8], rhs=sb_wo0[:, sl], start=True, stop=False)
        nc.tensor.matmul(ps[:], lhsT=on[:, 8:16], rhs=sb_wo1[:, sl], start=False, stop=True)
        if nchunk == 0:
            nc.vector.tensor_copy(out_f[:, sl], ps[:])
        else:
            nc.scalar.copy(out_f[:, sl], ps[:])
        (nc.sync if nchunk == 0 else nc.scalar).dma_start(out_p[:, sl], out_f[:, sl])


def _host_tables():
    """cos~/sin~ in transposed layout [128 (h,d), 4096 s] plus q-side tables."""
    inv_freq = 1.0 / (THETA ** (np.arange(0, ROT, 2, dtype=np.float64) / ROT))
    invf_rep = np.repeat(inv_freq, 2)  # [32]
    pos = np.arange(CL, dtype=np.float64)
    ang = invf_rep[:, None] * pos[None, :]  # [32 rot-d, 4096 s]
    cos_h = np.concatenate([np.cos(ang), np.ones((32, CL))], axis=0)  # [64, 4096]
    sin_h = np.concatenate([np.sin(ang), np.zeros((32, CL))], axis=0)
    cos_t = np.concatenate([cos_h, cos_h], axis=0).astype(np.float16)  # [128, 4096]
    sin_t = np.concatenate([sin_h, sin_h], axis=0).astype(np.float16)
    fq = 4096.0 * invf_rep
    cq_row = np.concatenate([np.cos(fq), np.ones(32)])  # per head [64]
    sq_row = np.concatenate([np.sin(fq), np.zeros(32)])
    cq_t = np.tile(np.concatenate([cq_row, cq_row]), (BS, 1)).astype(np.float32)
    sq_t = np.tile(np.concatenate([sq_row, sq_row]), (BS, 1)).astype(np.float32)
    return cos_t, sin_t, cq_t, sq_t


_NC = None


def _get_nc():
    global _NC
    if _NC is None:
        _NC = build_program()
    return _NC


def kernel(q, k_cache, v_cache, WQ_w, WQ_b, WK_w, WK_b, WV_w, WV_b, WO_w, WO_b,
           _trace=False, _tmpdir=None):
    q = np.asarray(q, dtype=np.float32)
    k16 = np.asarray(k_cache, dtype=np.float32).astype(np.float16)
    v8 = np.asarray(v_cache, dtype=np.float32).astype(ml_dtypes.float8_e3m4)
    cos_t, sin_t, cq_t, sq_t = _host_tables()
    q_t = np.ascontiguousarray(q.reshape(BS, D).T.astype(np.float16))
    id8 = np.eye(8, dtype=np.float32)
    id8f = np.eye(8, dtype=np.float16)
    hmask = np.zeros((128, 2), np.float16)
    hmask[0:64, 0] = 1.0
    hmask[64:128, 1] = 1.0

    in_maps = []
    for c in range(N_CORES):
        sl = slice(c * 128, (c + 1) * 128)
        hs = slice(c * H_PER_CORE, (c + 1) * H_PER_CORE)
        # K: [b,h,s,d] -> [b, (h d), s]
        kc = k16[:, hs].transpose(0, 1, 3, 2).reshape(BS, 128, CL)
        # V: [b,h,s,d] -> [b, p, (h c d)] with s = c*128 + p
        vc = v8[:, hs].reshape(BS, H_PER_CORE, 32, 128, HD)
        vc = vc.transpose(0, 3, 1, 2, 4).reshape(BS, 128, CL)
        in_maps.append({
            "k_c": np.ascontiguousarray(kc),
            "v_c": np.ascontiguousarray(vc),
            "q_t": q_t,
            "wqkv_t": np.ascontiguousarray(np.concatenate(
                [np.asarray(WQ_w, np.float32)[sl].T,
                 np.asarray(WK_w, np.float32)[sl].T,
                 np.asarray(WV_w, np.float32)[sl].T], axis=1).astype(np.float16)),
            "bqkv": np.ascontiguousarray(np.concatenate(
                [np.asarray(WQ_b, np.float32)[sl],
                 np.asarray(WK_b, np.float32)[sl],
                 np.asarray(WV_b, np.float32)[sl]]).reshape(1, 384).astype(np.float16)),
            "wo_t": np.ascontiguousarray(
                np.asarray(WO_w, np.float32)[:, sl].T.astype(np.float16)),
            "cos_t": cos_t, "sin_t": sin_t, "cq_t": cq_t, "sq_t": sq_t,
            "id8": id8, "id8f": id8f, "hmask": hmask,
        })

    nc = _get_nc()
    res = run_bass_kernel_spmd(nc, in_maps, list(range(N_CORES)),
                               trace=_trace, tmpdir=_tmpdir)
    partials = [np.asarray(res.results[c]["out_p"], dtype=np.float64)
                for c in range(N_CORES)]
    out = np.sum(partials, axis=0) + np.asarray(WO_b, np.float64)
    if _trace:
        kernel._last_results = res
    return out.reshape(BS, 1, D).astype(np.float32)


# revision 15
# speedup vs baseline: 1.4808x; 1.0378x over previous
"""Trainium2 Bass kernel for decode-step multi-head attention with RoPE
re-applied to the full KV cache (nn_MultiHeadAttention_50216757624897).

Sharding: 16 heads tensor-parallel across 8 cores (2 heads/core).
QKV weights split column-wise by head, KV cache split on the head dim,
out-proj row-parallel; partials summed on host (the unshard step).

Architecture (v3, transposed-K + 2-pass score fold):
 - K cache host-permuted to [b, (h,d), s] fp16: partitions carry the 2x64
   head-dims, the free dim carries all 4096 positions.
 - score[s,h] = sum_d k*cos~*u + sum_d k*sin~*v. Only the two products
   kc = k (.) cos~ and ks = k (.) sin~ are elementwise (DVE 2x / Pool);
   the (.) u / (.) v and the d-reduction fold into PE: per 128-position
   chunk, two accumulating matmuls with stationary lhsT = kc/ks chunks
   and rhs = tiny per-batch masks (hmask * u_b / * v_b), writing scores
   [128 positions, 2 heads] straight to PSUM. No E tile, no DVE reduce.
 - cos~ rows are 1 and sin~ rows are 0 on passthrough dims, so those need
   no separate handling (ks passthrough contributes v*0).
 - The new (current) token's K rotation cancels with Q's: score_new = qh.kh.
 - Softmax runs without max-subtraction (|score/8| < 3).
 - V cache host-permuted to [b, p, (h, c, d)] fp8-e3m4 (position = c*128+p):
   it is consumed only by PE A.V matmuls (fp8 full-rate), halving its HBM
   traffic; A.V contracts over partitions like the score layout.
 - DMA transfers overlap across issuing queues in the cost model; the kv
   stream alternates SP/Act and the cos/sin tables ship as halves on
   SWDGE/Act so both tables land by ~6us.
"""

import os
import sys
from contextlib import ExitStack

import numpy as np
import ml_dtypes

sys.path.insert(0, "/opt/trn_rl_repo")

import concourse.bass as bass
import concourse.bacc as bacc
import concourse.tile as tile
from concourse import mybir
from concourse.bass_types import AP
from concourse.bass_utils import run_bass_kernel_spmd

F32 = mybir.dt.float32
F16 = mybir.dt.float16
F8 = mybir.dt.float8e3
AF = mybir.ActivationFunctionType
AX = mybir.AxisListType
OP = mybir.AluOpType

BS, NH, HD, ROT, CL, D = 8, 16, 64, 32, 4096, 1024
THETA = 10000.0
N_CORES = 8
H_PER_CORE = NH // N_CORES  # 2
HALF = CL // 2

# ks half-products steered to DVE for these batches (engine balance knob)
KS_DVE = set(int(x) for x in os.environ.get("KS_DVE", "0,2,4,6").split(",") if x != "")


def _fap(t, off, dims):
    """AP over tile t with the tile's partition dim, extra free-dim spec."""
    b = t[:]
    return AP(tensor=b.tensor, offset=b.offset + off, ap=[list(b.ap[0])] + dims)


def _rotap(t, off):
    """[8, 2h, 16pairs] strided view of a [8,128] tile selecting pair elem
    `off` (0=even, 1=odd) of the rotary dims."""
    return _fap(t, off, [[64, 2], [2, 16]])


def build_program():
    nc = bacc.Bacc("TRN2", target_bir_lowering=False, debug=False)

    k_c = nc.dram_tensor("k_c", [BS, 128, CL], F16, kind="ExternalInput")
    v_c = nc.dram_tensor("v_c", [BS, 128, CL], F8, kind="ExternalInput")
    q_t = nc.dram_tensor("q_t", [D, BS], F16, kind="ExternalInput")
    wqkv_t = nc.dram_tensor("wqkv_t", [D, 384], F16, kind="ExternalInput")
    bqkv = nc.dram_tensor("bqkv", [1, 384], F16, kind="ExternalInput")
    wo_t = nc.dram_tensor("wo_t", [128, D], F16, kind="ExternalInput")
    cos_t = nc.dram_tensor("cos_t", [128, CL], F16, kind="ExternalInput")
    sin_t = nc.dram_tensor("sin_t", [128, CL], F16, kind="ExternalInput")
    cq_t = nc.dram_tensor("cq_t", [BS, 128], F32, kind="ExternalInput")
    sq_t = nc.dram_tensor("sq_t", [BS, 128], F32, kind="ExternalInput")
    id8 = nc.dram_tensor("id8", [8, 8], F32, kind="ExternalInput")
    id8f = nc.dram_tensor("id8f", [8, 8], F16, kind="ExternalInput")
    hmask = nc.dram_tensor("hmask", [128, 2], F16, kind="ExternalInput")
    out_p = nc.dram_tensor("out_p", [BS, D], F32, kind="ExternalOutput")

    with tile.TileContext(nc) as tc:
        with ExitStack() as ctx:
            _body(nc, tc, ctx, locals())
    nc.finalize()
    return nc


def _body(nc, tc, ctx, t):
    k_c, v_c = t["k_c"], t["v_c"]
    out_p = t["out_p"]

    const = ctx.enter_context(tc.tile_pool(name="const", bufs=1))
    small = ctx.enter_context(tc.tile_pool(name="small", bufs=1))

    # ---- constants. q/wqkv gate the q-chain (SP); table halves spread over
    # SWDGE + Act so both tables land by ~6us; kv stream follows.
    sb_qt = const.tile([128, 8, 8], F16, tag="qt")
    nc.sync.dma_start(sb_qt[:], t["q_t"].rearrange("(c p) b -> p c b", p=128))
    sb_wqkv = const.tile([128, 8, 384], F16, tag="wqkv")
    nc.sync.dma_start(sb_wqkv[:], t["wqkv_t"].rearrange("(c p) n -> p c n", p=128))

    sb_cos = const.tile([128, CL], F16, tag="cos")
    sb_sin = const.tile([128, CL], F16, tag="sin")
    nc.gpsimd.dma_start(sb_cos[:, 0:HALF], t["cos_t"][:, 0:HALF])
    nc.sync.dma_start(sb_cos[:, HALF:CL], t["cos_t"][:, HALF:CL])
    nc.gpsimd.dma_start(sb_sin[:, 0:HALF], t["sin_t"][:, 0:HALF])
    nc.sync.dma_start(sb_sin[:, HALF:CL], t["sin_t"][:, HALF:CL])

    sb_bqkv = const.tile([1, 384], F16, tag="bqkv")
    nc.scalar.dma_start(sb_bqkv[:], t["bqkv"][:, :])
    sb_cq = const.tile([BS, 128], F32, tag="cq")
    nc.scalar.dma_start(sb_cq[:], t["cq_t"][:, :])
    sb_sq = const.tile([BS, 128], F32, tag="sq")
    nc.scalar.dma_start(sb_sq[:], t["sq_t"][:, :])
    sb_id8 = const.tile([8, 8], F32, tag="id8")
    nc.scalar.dma_start(sb_id8[:], t["id8"][:, :])
    sb_id8f = const.tile([8, 8], F16, tag="id8f")
    nc.scalar.dma_start(sb_id8f[:], t["id8f"][:, :])
    sb_hmask = const.tile([128, 2], F16, tag="hmask")
    nc.scalar.dma_start(sb_hmask[:], t["hmask"][:, :])
    sb_wo0 = const.tile([64, 1024], F16, tag="wo0")
    nc.scalar.dma_start(sb_wo0[:], t["wo_t"][0:64, :])
    sb_wo1 = const.tile([64, 1024], F16, tag="wo1")
    nc.scalar.dma_start(sb_wo1[:], t["wo_t"][64:128, :])

    ones_p = const.tile([128, 1], F32, tag="ones_p")
    nc.vector.memset(ones_p[:], 1.0)
    ones_r8 = const.tile([1, 8], F16, tag="ones_r8")
    nc.vector.memset(ones_r8[:], 1.0)
    ones_r64 = const.tile([1, 64], F32, tag="ones_r64")
    nc.vector.memset(ones_r64[:], 1.0)

    # ---- q/k/v projection of the new token
    qtr_stack = ExitStack()
    psum_proj = qtr_stack.enter_context(tc.tile_pool(name="psum_proj", bufs=1, space="PSUM"))
    projs = small.tile([8, 384], F32, tag="projs")
    ps_qkv = psum_proj.tile([8, 384], F32, tag="ps_qkv")
    ps_q = ps_qkv[:, 0:128]
    for ci in range(8):
        nc.tensor.matmul(ps_q, lhsT=sb_qt[:, ci, :], rhs=sb_wqkv[:, ci, 0:128],
                         start=(ci == 0), stop=False, skip_group_check=True)
    nc.tensor.matmul(ps_q, lhsT=ones_r8[:], rhs=sb_bqkv[:, 0:128],
                     start=False, stop=True, skip_group_check=True)
    nc.scalar.copy(projs[:, 0:128], ps_q)
    ps_kv = ps_qkv[:, 128:384]
    for ci in range(8):
        nc.tensor.matmul(ps_kv, lhsT=sb_qt[:, ci, :], rhs=sb_wqkv[:, ci, 128:384],
                         start=False, stop=False, skip_group_check=True)
    nc.tensor.matmul(ps_kv, lhsT=ones_r8[:], rhs=sb_bqkv[:, 128:384],
                     start=False, stop=True, skip_group_check=True)
    nc.scalar.copy(projs[:, 128:384], ps_kv)
    qh, kh, vh = projs[:, 0:128], projs[:, 128:256], projs[:, 256:384]

    # ---- RoPE on q (full width: host tables carry [cos|1], [sin|0]); u and
    # v = G(u) side by side in one [8, 256] f16 tile.
    qrv = small.tile([8, 256], F16, tag="qrv")
    qr, vG = qrv[:, 0:128], qrv[:, 128:256]
    Hh = small.tile([8, 128], F32, tag="Hh")
    nc.vector.memset(Hh[:], 0.0)
    nc.vector.tensor_scalar_mul(_rotap(Hh, 0), _fap(ps_qkv, 1, [[64, 2], [2, 16]]), -1.0)
    nc.vector.tensor_copy(_rotap(Hh, 1), _fap(ps_qkv, 0, [[64, 2], [2, 16]]))
    t1q = small.tile([8, 128], F32, tag="t1q")
    nc.vector.tensor_mul(t1q[:], ps_q, sb_cq[:])
    t2q = small.tile([8, 128], F32, tag="t2q")
    nc.vector.tensor_mul(t2q[:], Hh[:], sb_sq[:])
    nc.vector.tensor_add(qr, t2q[:], t1q[:])
    # v = G(q_rot): pairs (x0,x1) -> (x1,-x0); zero elsewhere
    nc.vector.memset(vG, 0.0)
    nc.vector.tensor_copy(_fap(qrv, 128, [[64, 2], [2, 16]]),
                          _fap(qrv, 1, [[64, 2], [2, 16]]))
    nc.vector.tensor_scalar_mul(_fap(qrv, 129, [[64, 2], [2, 16]]),
                                _fap(qrv, 0, [[64, 2], [2, 16]]), -1.0)

    # ---- transpose u, v to per-partition layout [128 (h,d), 8 b]
    psum_tr = qtr_stack.enter_context(tc.tile_pool(name="psum_tr", bufs=1, space="PSUM"))
    uv_ps = psum_tr.tile([128, 16], F16, tag="uv_ps")
    nc.tensor.matmul(uv_ps[:, 0:8], lhsT=qr, rhs=sb_id8f[:], is_transpose=True,
                     start=True, stop=False, skip_group_check=True)
    nc.tensor.matmul(uv_ps[:, 8:16], lhsT=vG, rhs=sb_id8f[:], is_transpose=True,
                     start=False, stop=True, skip_group_check=True)
    u_T = small.tile([128, 8], F32, tag="u_T")
    nc.scalar.copy(u_T[:], uv_ps[:, 0:8])
    v_T = small.tile([128, 8], F32, tag="v_T")
    nc.scalar.copy(v_T[:], uv_ps[:, 8:16])

    # per-batch score-mask tiles: umask[:, (b,h)] = hmask[:, h] * u_T[:, b]
    um = small.tile([128, 16], F16, tag="um")
    vm = small.tile([128, 16], F16, tag="vm")
    for b in range(8):
        nc.vector.tensor_scalar(um[:, 2 * b:2 * b + 2], sb_hmask[:],
                                u_T[:, b:b + 1], None, OP.mult)
        nc.vector.tensor_scalar(vm[:, 2 * b:2 * b + 2], sb_hmask[:],
                                v_T[:, b:b + 1], None, OP.mult)

    # ---- new-token score: rotations cancel -> qh . kh
    sn = small.tile([8, 128], F32, tag="sn")
    nc.vector.tensor_mul(sn[:], qh, kh)
    scn = small.tile([8, 2], F32, tag="scn")
    nc.vector.reduce_sum(scn[:], _fap(sn, 0, [[64, 2], [1, 64]]), axis=AX.X)
    expn = small.tile([8, 2], F32, tag="expn")
    nc.scalar.activation(expn[:], scn[:], AF.Exp, scale=0.125)
    vhs = small.tile([8, 128], F32, tag="vhs")
    nc.vector.tensor_mul(_fap(vhs, 0, [[64, 2], [1, 64]]),
                         _fap(projs, 256, [[64, 2], [1, 64]]),
                         _fap(expn, 0, [[1, 2], [0, 64]]))

    qtr_stack.close()  # release proj/transpose PSUM banks for the loop pools

    # ---- main per-batch loop
    kpool = ctx.enter_context(tc.tile_pool(name="kpool", bufs=3))
    vpool = ctx.enter_context(tc.tile_pool(name="vpool", bufs=3))
    kcpool = ctx.enter_context(tc.tile_pool(name="kcpool", bufs=2))
    kspool = ctx.enter_context(tc.tile_pool(name="kspool", bufs=2))
    apool = ctx.enter_context(tc.tile_pool(name="apool", bufs=3))
    psum_sc = ctx.enter_context(tc.tile_pool(name="psum_sc", bufs=3, space="PSUM"))
    psum_r = ctx.enter_context(tc.tile_pool(name="psum_r", bufs=1, space="PSUM"))
    psum_wo = ctx.enter_context(tc.tile_pool(name="psum_wo", bufs=2, space="PSUM"))
    psum_main = ctx.enter_context(tc.tile_pool(name="psum_main", bufs=1, space="PSUM"))

    ov_ps = psum_main.tile([64, 16], F32, tag="ov")
    den_ps = psum_main.tile([1, 16], F32, tag="den")
    den_part = small.tile([128, 16], F32, tag="den_part")

    # init PSUM with the new-token contribution (transposes of vh*exp, exp)
    # NOTE: PSUM start=True zeroes the whole 2KB bank row, so only the FIRST
    # write into each psum tile may use start=True.
    for h in range(H_PER_CORE):
        nc.tensor.matmul(ov_ps[:, h * 8:(h + 1) * 8], lhsT=vhs[:, h * 64:(h + 1) * 64],
                         rhs=sb_id8[:], is_transpose=True, start=(h == 0), stop=False,
                         skip_group_check=True)
        nc.tensor.matmul(den_ps[:, h * 8:(h + 1) * 8], lhsT=expn[:, h:h + 1],
                         rhs=sb_id8[:], is_transpose=True, start=(h == 0), stop=False,
                         skip_group_check=True)

    def b_iter(b):
        kt = kpool.tile([128, CL], F16, tag="k")
        vt = vpool.tile([128, CL], F8, tag="v")
        kv_eng = nc.sync if b % 2 == 0 else nc.scalar
        kv_eng.dma_start(kt[:, 0:HALF], k_c[b][:, 0:HALF])
        kv_eng.dma_start(kt[:, HALF:CL], k_c[b][:, HALF:CL])
        kv_eng.dma_start(vt[:], v_c[b])

        # kc = k (.) cos~, ks = k (.) sin~, in col-halves
        kc = kcpool.tile([128, CL], F16, tag="kc")
        ks = kspool.tile([128, CL], F16, tag="ks")
        sc = psum_sc.tile([128, 64], F32, tag="sc", name=f"sc{b}")
        for half in range(2):
            lo, hi = half * HALF, (half + 1) * HALF
            nc.vector.tensor_mul(kc[:, lo:hi], kt[:, lo:hi], sb_cos[:, lo:hi])
            ks_eng = nc.vector if (b in KS_DVE and half == 0) else nc.gpsimd
            ks_eng.tensor_mul(ks[:, lo:hi], kt[:, lo:hi], sb_sin[:, lo:hi])
            for c in range(half * 16, half * 16 + 16):
                nc.tensor.matmul(sc[:, 2 * c:2 * c + 2],
                                 lhsT=kc[:, c * 128:(c + 1) * 128],
                                 rhs=um[:, 2 * b:2 * b + 2],
                                 start=(c == 0), stop=False, skip_group_check=True)
                nc.tensor.matmul(sc[:, 2 * c:2 * c + 2],
                                 lhsT=ks[:, c * 128:(c + 1) * 128],
                                 rhs=vm[:, 2 * b:2 * b + 2],
                                 start=False, stop=(c == 31), skip_group_check=True)

        # exp + denominators; at cols (32h + c) <- sc cols (2c + h)
        at = apool.tile([128, 64], F16, tag="at")
        for h in range(H_PER_CORE):
            col = h * 8 + b
            scv = _fap(sc, h, [[2, 32]])
            nc.scalar.activation(at[:, h * 32:(h + 1) * 32], scv,
                                 AF.Exp, scale=0.125,
                                 accum_out=den_part[:, col:col + 1])
            for c in range(32):
                nc.tensor.matmul(ov_ps[:, col:col + 1],
                                 lhsT=_fap(vt, h * 2048 + c * 64, [[1, 64]]),
                                 rhs=at[:, h * 32 + c:h * 32 + c + 1],
                                 start=False, stop=(c == 31), skip_group_check=True)

    for b in range(8):
        b_iter(b)

    # denominator: column-sum of per-partition exp sums + new-token init
    nc.tensor.matmul(den_ps[:], lhsT=ones_p[:], rhs=den_part[:],
                     start=False, stop=True, skip_group_check=True)

    # ---- normalize + out-projection
    ov_sb = small.tile([64, 16], F32, tag="ov_sb")
    nc.scalar.copy(ov_sb[:], ov_ps[:])
    r_row = small.tile([1, 16], F32, tag="r_row")
    nc.vector.reciprocal(r_row[:], den_ps[:])
    r_ps = psum_r.tile([64, 16], F32, tag="r")
    nc.tensor.matmul(r_ps[:], lhsT=ones_r64[:], rhs=r_row[:], start=True, stop=True)
    on = small.tile([64, 16], F16, tag="on")
    nc.vector.tensor_mul(on[:], ov_sb[:], r_ps[:])

    out_f = small.tile([8, 1024], F32, tag="out_f")
    for nchunk in range(2):
        sl = slice(nchunk * 512, (nchunk + 1) * 512)
        ps = psum_wo.tile([8, 512], F32, tag="wo", name=f"wo_ps{nchunk}")
        nc.tensor.matmul(ps[:], lhsT=on[:, 0:8], rhs=sb_wo0[:, sl], start=True, stop=False)
        nc.tensor.matmul(ps[:], lhsT=on[:, 8:16], rhs=sb_wo1[:, sl], start=False, stop=True)
        if nchunk == 0:
            nc.vector.tensor_copy(out_f[:, sl], ps[:])
        else:
            nc.scalar.copy(out_f[:, sl], ps[:])
        (nc.sync if nchunk == 0 else nc.scalar).dma_start(out_p[:, sl], out_f[:, sl])


def _host_tables():
    """cos~/sin~ in transposed layout [128 (h,d), 4096 s] plus q-side tables."""
    inv_freq = 1.0 / (THETA ** (np.arange(0, ROT, 2, dtype=np.float64) / ROT))
    invf_rep = np.repeat(inv_freq, 2)  # [32]
    pos = np.arange(CL, dtype=np.float64)
    ang = invf_rep[:, None] * pos[None, :]  # [32 rot-d, 4096 s]
    cos_h = np.concatenate([np.cos(ang), np.ones((32, CL))], axis=0)  # [64, 4096]
    sin_h = np.concatenate([np.sin(ang), np.zeros((32, CL))], axis=0)
    cos_t = np.concatenate([cos_h, cos_h], axis=0).astype(np.float16)  # [128, 4096]
    sin_t = np.concatenate([sin_h, sin_h], axis=0).astype(np.float16)
    fq = 4096.0 * invf_rep
    cq_row = np.concatenate([np.cos(fq), np.ones(32)])  # per head [64]
    sq_row = np.concatenate([np.sin(fq), np.zeros(32)])
    cq_t = np.tile(np.concatenate([cq_row, cq_row]), (BS, 1)).astype(np.float32)
    sq_t = np.tile(np.concatenate([sq_row, sq_row]), (BS, 1)).astype(np.float32)
    return cos_t, sin_t, cq_t, sq_t


_NC = None


def _get_nc():
    global _NC
    if _NC is None:
        _NC = build_program()
    return _NC


def kernel(q, k_cache, v_cache, WQ_w, WQ_b, WK_w, WK_b, WV_w, WV_b, WO_w, WO_b,
           _trace=False, _tmpdir=None):
    q = np.asarray(q, dtype=np.float32)
    k16 = np.asarray(k_cache, dtype=np.float32).astype(np.float16)
    v8 = np.asarray(v_cache, dtype=np.float32).astype(ml_dtypes.float8_e3m4)
    cos_t, sin_t, cq_t, sq_t = _host_tables()
    q_t = np.ascontiguousarray(q.reshape(BS, D).T.astype(np.float16))
    id8 = np.eye(8, dtype=np.float32)
    id8f = np.eye(8, dtype=np.float16)
    hmask = np.zeros((128, 2), np.float16)
    hmask[0:64, 0] = 1.0
    hmask[64:128, 1] = 1.0

    in_maps = []
    for c in range(N_CORES):
        sl = slice(c * 128, (c + 1) * 128)
        hs = slice(c * H_PER_CORE, (c + 1) * H_PER_CORE)
        # K: [b,h,s,d] -> [b, (h d), s]
        kc = k16[:, hs].transpose(0, 1, 3, 2).reshape(BS, 128, CL)
        # V: [b,h,s,d] -> [b, p, (h c d)] with s = c*128 + p
        vc = v8[:, hs].reshape(BS, H_PER_CORE, 32, 128, HD)
        vc = vc.transpose(0, 3, 1, 2, 4).reshape(BS, 128, CL)
        in_maps.append({
            "k_c": np.ascontiguousarray(kc),
            "v_c": np.ascontiguousarray(vc),
            "q_t": q_t,
            "wqkv_t": np.ascontiguousarray(np.concatenate(
                [np.asarray(WQ_w, np.float32)[sl].T,
                 np.asarray(WK_w, np.float32)[sl].T,
                 np.asarray(WV_w, np.float32)[sl].T], axis=1).astype(np.float16)),
            "bqkv": np.ascontiguousarray(np.concatenate(
                [np.asarray(WQ_b, np.float32)[sl],
                 np.asarray(WK_b, np.float32)[sl],
                 np.asarray(WV_b, np.float32)[sl]]).reshape(1, 384).astype(np.float16)),
            "wo_t": np.ascontiguousarray(
                np.asarray(WO_w, np.float32)[:, sl].T.astype(np.float16)),
            "cos_t": cos_t, "sin_t": sin_t, "cq_t": cq_t, "sq_t": sq_t,
            "id8": id8, "id8f": id8f, "hmask": hmask,
        })

    nc = _get_nc()
    res = run_bass_kernel_spmd(nc, in_maps, list(range(N_CORES)),
                               trace=_trace, tmpdir=_tmpdir)
    partials = [np.asarray(res.results[c]["out_p"], dtype=np.float64)
                for c in range(N_CORES)]
    out = np.sum(partials, axis=0) + np.asarray(WO_b, np.float64)
    if _trace:
        kernel._last_results = res
    return out.reshape(BS, 1, D).astype(np.float32)
